# revision 1
# baseline (speedup 1.0000x reference)
"""Causal self-attention Trainium2 kernel (B=4, T=2048, E=1024, H=16, D=64).

Sharding: 8 cores = batch(4) x head-group(2). Each core computes the full
attention for 8 heads of one batch element plus its half of the output
projection; the host sums the two out-proj partials per batch element.

v2 dataflow (per core, all matmul operands bf16, PSUM f32):
  - Host pre-transposes/bf16-casts x and the weights so contraction dims
    land on partitions: xT [E,T], wqkvT [E,1536], woT [512,E]. All weights
    resident in SBUF for the whole kernel.
  - Projection runs per T-quarter and is interleaved with attention of the
    previous query tile so the PE never idles (keeps the HAM clock-gate at
    8/8) and ScalarE exp overlaps projection matmuls.
  - S^T chunks [128kv, 512q]: the two heads of a pair are issued as
    row-group-tiled matmul pairs (tile_position (0,0)/(64,0)) that run
    concurrently in the PE array. Diagonal chunks only compute live
    columns.
  - exp on ScalarE (scale=1/sqrt(D)) -> bf16; causal mask applied via a
    DVE multiply with a precomputed [128,640] zeros|tril constant.
  - y^T accumulation [65, 512] per head with lhsT = V_aug (ones column
    carries the softmax denominator through the PV matmul).
  - Denominator rows are staged to a [32,512] SBUF tile (DVE copy to
    partition 64 + partition-shifting SBUF DMA), reciprocal'd in one DVE
    reciprocal_approx_fast per query tile, gpsimd-broadcast, and applied
    as one in-place [128,512] DVE multiply per (pair, tile). ScalarE
    runs Exp only - no activation-table reloads anywhere.
  - Out-projection for tile j is emitted after attention tile j+1 so its
    PSUM matmuls never stall the PE stream; partials DMA out as bf16 and
    the host sums the two head-group halves in f32.
"""

import numpy as np
import ml_dtypes

import concourse.bass as bass
import concourse.bacc as bacc
import concourse.mybir as mybir
import concourse.tile as tile
from concourse import bass_utils

f32 = mybir.dt.float32
bf16 = mybir.dt.bfloat16
FP = mybir.dt.float32  # psum dtype

P = 128
B, T, E = 4, 2048, 1024
H, D = 16, 64
HPC = H // 2            # heads per core = 8
NE = E // P             # 8 e-chunks
NTT = T // P            # 16 kv chunks
NQ = T // 512           # 4 query tiles of 512
SCALE = 1.0 / np.sqrt(D)

Exp = mybir.ActivationFunctionType.Exp
MULT = mybir.AluOpType.mult
IS_GE = mybir.AluOpType.is_ge

_CACHE = {}


def build(**opts):
    nc = bacc.Bacc("TRN2", target_bir_lowering=False, debug=False, num_devices=8)

    xT_d = nc.dram_tensor("xT", [E, T], bf16, kind="ExternalInput")
    xP0_d = nc.dram_tensor("xP0", [4, NE, P, P], bf16, kind="ExternalInput")
    wvP_d = nc.dram_tensor("wvP", [P, NE, 512], bf16, kind="ExternalInput")
    wqkP_d = nc.dram_tensor("wqkP", [P, 8, NE, P], bf16, kind="ExternalInput")
    woP_d = nc.dram_tensor("woP", [P, 4, E], bf16, kind="ExternalInput")
    mask_d = nc.dram_tensor("mask", [P, 640], bf16, kind="ExternalInput")
    out_d = nc.dram_tensor("out", [T, E], bf16, kind="ExternalOutput")

    with tile.TileContext(nc) as tc:
        build_body(tc, xT_d, xP0_d, wvP_d, wqkP_d, woP_d, mask_d, out_d,
                   **opts)
    nc.compile()
    return nc


def build_body(tc, xT_d, xP0_d, wvP_d, wqkP_d, woP_d, mask_d, out_d,
               pss_bufs=2, psy_bufs=2):
    nc = tc.nc

    from contextlib import ExitStack
    with ExitStack() as top:
        per = top.enter_context(tc.tile_pool(name="per", bufs=1))

        qk_sb = per.tile([P, 8, T], bf16)            # chunks 0-3: Q^T, 4-7: K^T
        v_sb = per.tile([P, NTT, HPC, D + 1], bf16)  # [kv_p, kv_chunk, head, d|1]
        yt_sb = per.tile([P, 4, T], bf16)            # [f%128, f//128, q]
        wv_sb = per.tile([P, NE, 512], bf16)         # V-proj weights
        wqk_sb = per.tile([P, 8, NE, P], bf16)       # QK-proj weights per f-chunk
        wo_sb = per.tile([P, 4, E], bf16)            # out-proj weights
        mask_sb = per.tile([P, 640], bf16)           # zeros(512) | tril(128)
        # softmax denominators + reciprocals, one [8,512] tile per query
        # tile (ISA ops require APs that start at partition 0); j=3 is
        # normalized per-c to shorten the tail, so it gets [2,512] tiles
        l_js = [per.tile([8, 512], f32, name=f"l_sb{j}") for j in range(3)]
        rc_js = [per.tile([8, 512], f32, name=f"rc_sb{j}") for j in range(3)]
        l_c3 = [per.tile([2, 512], f32, name=f"l_c3_{c}") for c in range(4)]
        rc_c3 = [per.tile([2, 512], f32, name=f"rc_c3_{c}") for c in range(4)]
        # self-managed ptt ring: persistent tiles, zeroed once so the causal
        # mask-multiply never sees uninitialized data (NaN*0=NaN) in the
        # stale columns the diagonal chunks skip
        ptt_ring = [per.tile([P, 1024], bf16, name=f"ptt{k}")
                    for k in range(4)]
        for pt in ptt_ring:
            nc.vector.memset(pt, 0.0)
        ptt_ctr = [0]

        # --------- prologue DMAs, spread across engine queues so they run
        # in parallel and the first V matmul can start ~1-2us in ----------
        xpool = top.enter_context(tc.tile_pool(name="xpool", bufs=3))
        xts = {}

        def load_x_quarter(th, engine=None):
            eng = engine or nc.gpsimd
            for e in range(NE):
                xt = xpool.tile([P, 512], bf16, tag=f"xt{e}")
                eng.dma_start(
                    xt, xT_d[e * P:(e + 1) * P, th * 512:(th + 1) * 512])
                xts[(e, th)] = xt

        # warm the GpSimd custom-op ucode library during the DMA wait (the
        # first partition_broadcast otherwise costs ~7us mid-kernel)
        nc.gpsimd.partition_broadcast(rc_js[0][0:4, :], l_js[0][0:1, :])
        # quarter 0 in host-prechunked [128,128] pieces, tti-major, on the
        # (empty) Sync queue: the first V-proj chain needs col slice 0:128
        # of every e; all transfers fully contiguous
        for e in range(NE):
            xts[(e, 0)] = xpool.tile([P, 512], bf16, tag=f"xt{e}",
                                     name=f"xt{e}_0")
        for tti in range(4):
            for e in range(NE):
                nc.sync.dma_start(
                    xts[(e, 0)][:, tti * P:(tti + 1) * P],
                    xP0_d[tti, e])
        nc.gpsimd.dma_start(wv_sb[:, 0:4], wvP_d[:, 0:4, :])
        nc.scalar.dma_start(wv_sb[:, 4:8], wvP_d[:, 4:8, :])
        nc.scalar.dma_start(wqk_sb[:, 0:2], wqkP_d[:, 0:2])
        nc.sync.dma_start(wqk_sb[:, 2:4], wqkP_d[:, 2:4])
        nc.gpsimd.dma_start(wqk_sb[:, 4:6], wqkP_d[:, 4:6])
        nc.scalar.dma_start(wqk_sb[:, 6:8], wqkP_d[:, 6:8])
        nc.sync.dma_start(mask_sb, mask_d[:, :])
        nc.scalar.dma_start(wo_sb, woP_d[:, :, :])
        load_x_quarter(1)
        load_x_quarter(2)

        # pool creation order: psp LAST so it sits on top of the PSUM stack
        # and can be swapped for the out-proj pool after projections end
        drn = top.enter_context(tc.tile_pool(name="drn", bufs=2))
        nrm = top.enter_context(tc.tile_pool(name="nrm", bufs=2))
        ost = top.enter_context(tc.tile_pool(name="ost", bufs=2))
        pss = top.enter_context(
            tc.tile_pool(name="pss", bufs=pss_bufs, space="PSUM"))
        psy = top.enter_context(
            tc.tile_pool(name="psy", bufs=psy_bufs, space="PSUM"))
        psp_ctx = ExitStack()
        psp = psp_ctx.enter_context(
            tc.tile_pool(name="psp", bufs=2, space="PSUM"))
        pools = {}

        def piece_v(th, tti):
            # one V-projection chain: v_sb chunk tt, natural layout
            tt = th * 4 + tti
            ps = psp.tile([P, 512], FP, tag="pp")
            for e in range(NE):
                nc.tensor.matmul(
                    ps,
                    lhsT=xts[(e, th)][:, tti * P:(tti + 1) * P],
                    rhs=wv_sb[:, e, :],
                    start=(e == 0), stop=(e == NE - 1))
            nc.vector.tensor_copy(
                v_sb[:, tt, :, 0:D],
                ps.rearrange("p (h d) -> p h d", h=HPC))
            if tti == 3:
                # ones column for this quarter (never keeps in_: cond<0)
                ov = v_sb[:, th * 4:(th + 1) * 4, :, D:D + 1]
                iv = v_sb[:, th * 4:(th + 1) * 4, :, 0:1]
                nc.gpsimd.affine_select(
                    ov, iv, pattern=[[0, 4], [0, HPC], [0, 1]],
                    compare_op=IS_GE, fill=1.0, base=-1,
                    channel_multiplier=0)

        def piece_qk(th, ft):
            # one QK-projection chain: qk_sb chunk ft, [f, 512] layout
            ps = psp.tile([P, 512], FP, tag="pp")
            for e in range(NE):
                nc.tensor.matmul(
                    ps,
                    lhsT=wqk_sb[:, ft, e, :],
                    rhs=xts[(e, th)][:, :],
                    start=(e == 0), stop=(e == NE - 1))
            nc.vector.tensor_copy(
                qk_sb[:, ft, th * 512:(th + 1) * 512], ps)

        def proj_pieces(th):
            return ([lambda tti=tti: piece_v(th, tti) for tti in range(4)]
                    + [lambda ft=ft: piece_qk(th, ft)
                       for ft in (0, 4, 1, 5, 2, 6, 3, 7)])

        def piece_out(tt, half):
            # half an out-proj row tile: own PSUM bank, never couples with
            # the attention pipeline's S ring
            pso = pools["pso"]
            po = pso.tile([P, 512], FP, tag="po")
            for c2 in range(4):
                nc.tensor.matmul(
                    po,
                    lhsT=yt_sb[:, c2, tt * P:(tt + 1) * P],
                    rhs=wo_sb[:, c2, half * 512:(half + 1) * 512],
                    start=(c2 == 0), stop=(c2 == 3))
            st = ost.tile([P, 512], bf16, tag="st")
            nc.vector.tensor_copy(st, po)
            nc.sync.dma_start(
                out_d[tt * P:(tt + 1) * P, half * 512:(half + 1) * 512], st)

        def att_block(c, j, fillers=None, every=4):
            """Attention for head pair (2c, 2c+1), query tile j.

            Pops one filler (independent PE work) every `every` chunks so
            the PE stays busy while ScalarE works through the exps."""
            jsl = slice(j * 512, (j + 1) * 512)
            nkv = 4 * j + 4
            yps = [psy.tile([D + 1, 512], FP, tag="y", name=f"yps_{c}_{j}_{k}")
                   for k in range(2)]
            for i in range(nkv):
                off = i - 4 * j
                q0 = max(0, 128 * off)
                spt = pss.tile([P, 1024], FP, tag="s")
                for hh in range(2):
                    p0 = 64 * hh
                    nc.tensor.matmul(
                        spt[:, hh * 512 + q0:(hh + 1) * 512],
                        lhsT=qk_sb[p0:p0 + 64, 4 + c, i * P:(i + 1) * P],
                        rhs=qk_sb[p0:p0 + 64, c, j * 512 + q0:(j + 1) * 512],
                        start=True, stop=True,
                        tile_position=(p0, 0))
                ptt = ptt_ring[ptt_ctr[0] % 4]
                ptt_ctr[0] += 1
                if off < 0:
                    nc.scalar.activation(ptt, spt, Exp, scale=float(SCALE))
                else:
                    pv = ptt.rearrange("p (h q) -> p h q", h=2)
                    sv = spt.rearrange("p (h q) -> p h q", h=2)
                    nc.scalar.activation(pv[:, :, q0:512], sv[:, :, q0:512],
                                         Exp, scale=float(SCALE))
                    # causal mask: zero stale cols [0,q0) + triangle
                    # [q0,q0+128) via mask constant (zeros(512)|tril(128))
                    for hx in range(2):
                        nc.vector.tensor_tensor(
                            pv[:, hx, 0:q0 + P], pv[:, hx, 0:q0 + P],
                            mask_sb[:, 512 - q0:640], MULT)
                for hh in range(2):
                    nc.tensor.matmul(
                        yps[hh],
                        lhsT=v_sb[:, i, 2 * c + hh, :],
                        rhs=ptt[:, hh * 512:(hh + 1) * 512],
                        start=(i == 0), stop=(i == nkv - 1))
                if fillers and (i + 1) % every == 0:
                    fillers.pop(0)()
            # drain: unnormalized y -> yt_sb lower half / tmpb (upper half
            # staged until norm), denominators -> l rows (they ride
            # partition 64 of sc, then partition-shifting SBUF DMAs place
            # them on partition-0-based tiles for the batched recip)
            sc = drn.tile([D + 1, 1024], f32, tag="sc")
            nc.vector.tensor_copy(sc[D:D + 1, 0:512], yps[0][D:D + 1, :])
            nc.vector.tensor_copy(sc[D:D + 1, 512:1024], yps[1][D:D + 1, :])
            if j == 3:
                nc.sync.dma_start(l_c3[c][0:1, :], sc[D:D + 1, 0:512])
                nc.sync.dma_start(l_c3[c][1:2, :], sc[D:D + 1, 512:1024])
            else:
                nc.sync.dma_start(l_js[j][2 * c:2 * c + 1, :],
                                  sc[D:D + 1, 0:512])
                nc.sync.dma_start(l_js[j][2 * c + 1:2 * c + 2, :],
                                  sc[D:D + 1, 512:1024])
            nc.vector.tensor_copy(yt_sb[0:D, c, jsl], yps[0][0:D, :])
            tmpb = drn.tile([D, 512], bf16, tag="tmpb", bufs=5)
            nc.vector.tensor_copy(tmpb, yps[1][0:D, :])
            tmpbs[(c, j)] = tmpb

        def _norm_pair(c, j, rcb, k, eng=None):
            """broadcast rcb rows [k, k+1] (partition-0 sources) and
            normalize pair (c, j): lower yt half in place, upper half via
            tmpn then the partition-shifting DMA into yt. Multiplies run
            on GpSimd by default so they never block the DVE FIFO (which
            carries the attention mask-multiplies); the latency-critical
            tail norm passes eng=nc.vector instead."""
            eng = eng or nc.gpsimd
            jsl = slice(j * 512, (j + 1) * 512)
            bc0 = nrm.tile([D, 512], bf16, tag="bc", bufs=3)
            nc.gpsimd.partition_broadcast(bc0, rcb[0:1, k, :])
            eng.tensor_tensor(
                yt_sb[0:D, c, jsl], yt_sb[0:D, c, jsl], bc0, MULT)
            bc1 = nrm.tile([D, 512], bf16, tag="bc", bufs=3)
            nc.gpsimd.partition_broadcast(bc1, rcb[0:1, k + 1, :])
            tmpn = nrm.tile([D, 512], bf16, tag="tmpn")
            eng.tensor_tensor(tmpn, tmpbs[(c, j)], bc1, MULT)
            nc.sync.dma_start(yt_sb[64:128, c, jsl], tmpn)

        def norm(j):
            """Reciprocal + broadcast + normalize of yt tile j (j<3).

            partition_broadcast needs its source on partition 0, so the 8
            reciprocal rows of tile j are first gathered into a single
            partition-0 tile with one SBUF DMA (cast to bf16 on the way)."""
            nc.vector.reciprocal_approx_fast(rc_js[j][:, :], l_js[j][:, :])
            rcrow = nrm.tile([1, 8, 512], f32, tag="rcrow", bufs=1)
            nc.sync.dma_start(rcrow, rc_js[j][:, :])
            rcb = nrm.tile([1, 8, 512], bf16, tag="rcb", bufs=1)
            nc.vector.tensor_copy(rcb, rcrow)
            for c in range(4):
                _norm_pair(c, j, rcb, 2 * c)

        def norm_c3(c, eng=None):
            """Per-c normalization for query tile 3 (shortens the tail)."""
            nc.vector.reciprocal_approx_fast(rc_c3[c][:, :], l_c3[c][:, :])
            rcrow = nrm.tile([1, 2, 512], f32, tag="rcrowc", bufs=1)
            nc.sync.dma_start(rcrow, rc_c3[c][:, :])
            rcb = nrm.tile([1, 2, 512], bf16, tag="rcbc", bufs=1)
            nc.vector.tensor_copy(rcb, rcrow)
            _norm_pair(c, 3, rcb, 0, eng=eng)

        # ------------------- emission schedule -------------------
        # Attention tile j is ScalarE(exp)-paced; fillers (projection of
        # quarter j+1, out-projections) are popped between chunks so the
        # PE never idles long enough to drop HAM to 4/8. norm(j) is
        # emitted one block into tile j+1 so its reciprocal never stalls
        # the DVE FIFO waiting on the l-row DMAs.
        tmpbs = {}
        for tti in range(4):
            piece_v(0, tti)
        for ft in (0, 4, 1, 5, 2, 6, 3, 7):
            piece_qk(0, ft)

        F = proj_pieces(1)
        for c in range(4):
            att_block(c, 0, F, every=1)
        while F:
            F.pop(0)()
        load_x_quarter(3)

        F = proj_pieces(2)
        att_block(0, 1, F, every=2)
        norm(0)
        for c in range(1, 4):
            att_block(c, 1, F, every=2)
        while F:
            F.pop(0)()

        F = proj_pieces(3)
        att_block(0, 2, F, every=3)
        norm(1)
        for c in range(1, 4):
            att_block(c, 2, F, every=3)
        while F:
            F.pop(0)()

        # projections done: swap the proj PSUM banks for the out-proj pool
        psp_ctx.close()
        pools["pso"] = top.enter_context(
            tc.tile_pool(name="pso", bufs=2, space="PSUM"))

        outs = [lambda tt=tt, half=half: piece_out(tt, half)
                for tt in range(12) for half in range(2)]
        att_block(0, 3, outs[0:6], every=2)
        norm(2)
        att_block(1, 3, outs[6:12] + [lambda: norm_c3(0)], every=2)
        att_block(2, 3, outs[12:18] + [lambda: norm_c3(1)], every=2)
        att_block(3, 3, outs[18:24] + [lambda: norm_c3(2)], every=2)
        norm_c3(3, eng=nc.vector)
        for tt in range(12, 16):
            piece_out(tt, 0)
            piece_out(tt, 1)


def _shard_inputs(x, w_qkv, w_out):
    mask = np.zeros((P, 640), dtype=np.float32)
    mask[:, 512:640] = np.tril(np.ones((P, P), dtype=np.float32)).T
    mask = mask.astype(ml_dtypes.bfloat16)
    in_maps = []
    for core in range(8):
        b, hg = core // 2, core % 2
        sl = slice(hg * 512, (hg + 1) * 512)
        wq = w_qkv[0:1024][sl]
        wk = w_qkv[1024:2048][sl]
        wv = w_qkv[2048:3072][sl]
        wqkvT = np.concatenate([wq, wk, wv], axis=0).T  # [E, 1536]
        wvP = wqkvT[:, 1024:1536].reshape(NE, P, 512).transpose(1, 0, 2)
        wqkP = wqkvT[:, 0:1024].reshape(NE, P, 8, P).transpose(1, 2, 0, 3)
        woT = w_out[:, sl].T  # [512, E]
        woP = woT.reshape(4, P, E).transpose(1, 0, 2)
        xT = np.ascontiguousarray(x[b].T)  # [E, T]
        xP0 = xT[:, 0:512].reshape(NE, P, 4, P).transpose(2, 0, 1, 3)
        cvt = lambda a: np.ascontiguousarray(a).astype(ml_dtypes.bfloat16)
        in_maps.append({
            "xT": cvt(xT),
            "xP0": cvt(xP0),
            "wvP": cvt(wvP),
            "wqkP": cvt(wqkP),
            "woP": cvt(woP),
            "mask": mask,
        })
    return in_maps





def kernel(x, w_qkv, w_out, _trace=False):
    x = np.asarray(x, dtype=np.float32)
    w_qkv = np.asarray(w_qkv, dtype=np.float32)
    w_out = np.asarray(w_out, dtype=np.float32)

    if "nc" not in _CACHE:
        _CACHE["nc"] = build()
    nc = _CACHE["nc"]

    in_maps = _shard_inputs(x, w_qkv, w_out)
    res = bass_utils.run_bass_kernel_spmd(
        nc, in_maps, core_ids=list(range(8)), trace=_trace)
    kernel.last_result = res

    out = np.empty((B, T, E), dtype=np.float32)
    for b in range(B):
        out[b] = (res.results[2 * b]["out"].astype(np.float32)
                  + res.results[2 * b + 1]["out"].astype(np.float32))
    return out



# revision 27
# speedup vs baseline: 1.2083x; 1.2083x over previous
"""Causal self-attention Trainium2 kernel (B=4, T=2048, E=1024, H=16, D=64).

Sharding: 8 cores = batch(4) x head-group(2). Each core computes the full
attention for 8 heads of one batch element plus its half of the output
projection; the host sums the two out-proj partials per batch element.

v3 dataflow (per core, all matmul operands bf16, PSUM f32):
  - x^T [E,T] lives fully in SBUF, loaded with 4 large DMAs (one per
    T-quarter) on two queues so the first V-proj chain starts ~4us in.
  - Projection chains (V then QK per quarter) are interleaved into the
    attention stream as fillers so the PE never idles while ScalarE works
    through the exps; out-projection row tiles become fillers as soon as
    their query quarter is normalized.
  - S^T chunks [128kv, 2x512q]: two heads of a pair issued as row-group
    tiled matmul pairs (tile_position (0,0)/(64,0)) running concurrently.
    Diagonal chunks only compute live columns; exp on ScalarE -> bf16;
    causal mask via DVE multiply with a [128,640] zeros|tril constant.
  - y^T accumulation [65,512] per head, lhsT = V_aug (ones column carries
    the softmax denominator through the PV matmul).
  - Drain per (pair, tile): DVE casts y rows to yt/tmpb, tmpb is DMA'd
    into yt's upper partitions immediately (not norm-gated), DVE
    reciprocal runs directly on the PSUM denominator rows (partition 64),
    one cast + one tiny DMA lands both rcp rows on a partition-0 table.
  - Norm per (pair, tile), emitted ~one block later so every op's deps are
    met when it reaches its engine FIFO: 2 gpsimd partition_broadcasts +
    2 in-place DVE multiplies on yt. No cross-engine convoys.
  - ScalarE exp table is pre-warmed during the prologue DMAs.
"""

import numpy as np
import ml_dtypes

import concourse.bass as bass
import concourse.bacc as bacc
import concourse.mybir as mybir
import concourse.tile as tile
from concourse import bass_utils

f32 = mybir.dt.float32
bf16 = mybir.dt.bfloat16
FP = mybir.dt.float32  # psum dtype

P = 128
B, T, E = 4, 2048, 1024
H, D = 16, 64
HPC = H // 2            # heads per core = 8
NE = E // P             # 8 e-chunks
NTT = T // P            # 16 kv chunks
NQ = T // 512           # 4 query tiles of 512
SCALE = 1.0 / np.sqrt(D)

Exp = mybir.ActivationFunctionType.Exp
MULT = mybir.AluOpType.mult
IS_GE = mybir.AluOpType.is_ge

_CACHE = {}


def build(**opts):
    nc = bacc.Bacc("TRN2", target_bir_lowering=False, debug=False, num_devices=8)

    xT_d = nc.dram_tensor("xT", [P, NE, T], bf16, kind="ExternalInput")
    wvP_d = nc.dram_tensor("wvP", [P, NE, 512], bf16, kind="ExternalInput")
    wqkP_d = nc.dram_tensor("wqkP", [P, 8, NE, P], bf16, kind="ExternalInput")
    woP_d = nc.dram_tensor("woP", [P, 4, E], bf16, kind="ExternalInput")
    mask_d = nc.dram_tensor("mask", [P, 640], bf16, kind="ExternalInput")
    out_d = nc.dram_tensor("out", [T, E], bf16, kind="ExternalOutput")
    dbgL_d = dbgB_d = None
    if opts.pop("debug_rcp", False):
        dbgL_d = nc.dram_tensor("dbgL", [16, 1024], f32,
                                kind="ExternalOutput")
        dbgB_d = nc.dram_tensor("dbgB", [16, 1024], bf16,
                                kind="ExternalOutput")

    with tile.TileContext(nc) as tc:
        build_body(tc, xT_d, wvP_d, wqkP_d, woP_d, mask_d, out_d,
                   dbgL_d=dbgL_d, dbgB_d=dbgB_d, **opts)
    nc.compile()
    return nc


def build_body(tc, xT_d, wvP_d, wqkP_d, woP_d, mask_d, out_d,
               pss_bufs=2, psy_bufs=2, norm_mode="full",
               dbgL_d=None, dbgB_d=None):
    nc = tc.nc

    from contextlib import ExitStack
    with ExitStack() as top:
        per = top.enter_context(tc.tile_pool(name="per", bufs=1))

        qk_sb = per.tile([P, 8, T], bf16)            # chunks 0-3: Q^T, 4-7: K^T
        v_sb = per.tile([P, NTT, HPC, D + 1], bf16)  # [kv_p, kv_chunk, head, d|1]
        yt_sb = per.tile([P, 4, T], bf16)            # [f%128, f//128, q]
        x_sb = per.tile([P, NE, T], bf16)            # x^T resident [p, e, t]
        wv_sb = per.tile([P, NE, 512], bf16)         # V-proj weights
        wqk_sb = per.tile([P, 8, NE, P], bf16)       # QK-proj weights per f-chunk
        wo_sb = per.tile([P, 4, E], bf16)            # out-proj weights
        mask_sb = per.tile([P, 640], bf16)           # zeros(512) | tril(128)
        warm_sb = per.tile([4, 512], bf16)           # gpsimd ucode warmup dst
        # self-managed ptt ring: persistent tiles, zeroed once so the causal
        # mask-multiply never sees uninitialized data (NaN*0=NaN) in the
        # stale columns the diagonal chunks skip
        ptt_ring = [per.tile([P, 1024], bf16, name=f"ptt{k}")
                    for k in range(4)]
        for pt in ptt_ring:
            nc.vector.memset(pt, 0.0)
        ptt_ctr = [0]

        # --------- prologue DMAs: 4 big x loads + weights, spread across
        # queues; warm the gpsimd ucode library and the ScalarE exp table
        # during the transfer ----------
        nc.gpsimd.partition_broadcast(warm_sb, ptt_ring[1][0:1, 0:512])
        # x quarter 0 first (sync queue) so V-proj can start asap
        nc.sync.dma_start(x_sb[:, :, 0:512], xT_d[:, :, 0:512])
        nc.scalar.dma_start(wv_sb, wvP_d[:, :, :])
        nc.scalar.dma_start(wqk_sb[:, 0:4], wqkP_d[:, 0:4])
        nc.scalar.dma_start(wqk_sb[:, 4:8], wqkP_d[:, 4:8])
        nc.scalar.dma_start(wo_sb, woP_d[:, :, :])
        nc.sync.dma_start(mask_sb, mask_d[:, :])
        # warm the exp table-set while DMAs fly (first ACT pays ~2.7us)
        nc.scalar.activation(ptt_ring[0][:, 0:8], ptt_ring[0][:, 0:8],
                             Exp, scale=1.0)
        for th in range(1, 4):
            nc.gpsimd.dma_start(
                x_sb[:, :, th * 512:(th + 1) * 512],
                xT_d[:, :, th * 512:(th + 1) * 512])

        # pool creation order: psp LAST so it sits on top of the PSUM stack
        # and can be swapped for the out-proj pool after projections end
        drn = top.enter_context(tc.tile_pool(name="drn", bufs=2))
        nrm = top.enter_context(tc.tile_pool(name="nrm", bufs=3))
        ost = top.enter_context(tc.tile_pool(name="ost", bufs=2))
        pss = top.enter_context(
            tc.tile_pool(name="pss", bufs=pss_bufs, space="PSUM"))
        psy = top.enter_context(
            tc.tile_pool(name="psy", bufs=psy_bufs, space="PSUM"))
        psp_ctx = ExitStack()
        psp = psp_ctx.enter_context(
            tc.tile_pool(name="psp", bufs=2, space="PSUM"))
        pools = {}
        done = set()
        lps = {}

        def piece_v(th, tti):
            # one V-projection chain: v_sb chunk tt, natural layout
            if ("v", th, tti) in done:
                return
            done.add(("v", th, tti))
            tt = th * 4 + tti
            ps = psp.tile([P, 512], FP, tag="pp")
            for e in range(NE):
                nc.tensor.matmul(
                    ps,
                    lhsT=x_sb[:, e, tt * P:(tt + 1) * P],
                    rhs=wv_sb[:, e, :],
                    start=(e == 0), stop=(e == NE - 1))
            nc.vector.tensor_copy(
                v_sb[:, tt, :, 0:D],
                ps.rearrange("p (h d) -> p h d", h=HPC))
            if tti == 3:
                # ones column for this quarter (never keeps in_: cond<0)
                ov = v_sb[:, th * 4:(th + 1) * 4, :, D:D + 1]
                iv = v_sb[:, th * 4:(th + 1) * 4, :, 0:1]
                nc.gpsimd.affine_select(
                    ov, iv, pattern=[[0, 4], [0, HPC], [0, 1]],
                    compare_op=IS_GE, fill=1.0, base=-1,
                    channel_multiplier=0)

        def piece_qk(th, ft):
            # one QK-projection chain: qk_sb chunk ft, [f, 512] layout
            if ("qk", th, ft) in done:
                return
            done.add(("qk", th, ft))
            ps = psp.tile([P, 512], FP, tag="pp")
            for e in range(NE):
                nc.tensor.matmul(
                    ps,
                    lhsT=wqk_sb[:, ft, e, :],
                    rhs=x_sb[:, e, th * 512:(th + 1) * 512],
                    start=(e == 0), stop=(e == NE - 1))
            nc.vector.tensor_copy(
                qk_sb[:, ft, th * 512:(th + 1) * 512], ps)

        def piece_out(tt, half):
            # half an out-proj row tile: own PSUM bank, never couples with
            # the attention pipeline's S ring
            pso = pools["pso"]
            po = pso.tile([P, 512], FP, tag="po")
            for c2 in range(4):
                nc.tensor.matmul(
                    po,
                    lhsT=yt_sb[:, c2, tt * P:(tt + 1) * P],
                    rhs=wo_sb[:, c2, half * 512:(half + 1) * 512],
                    start=(c2 == 0), stop=(c2 == 3))
            st = ost.tile([P, 512], bf16, tag="st")
            nc.vector.tensor_copy(st, po)
            nc.sync.dma_start(
                out_d[tt * P:(tt + 1) * P, half * 512:(half + 1) * 512], st)

        def att_block(c, j, fillers=None, every=4):
            """Attention for head pair (2c, 2c+1), query tile j.

            Pops one filler (independent work) every `every` chunks so the
            PE stays busy while ScalarE works through the exps."""
            jsl = slice(j * 512, (j + 1) * 512)
            nkv = 4 * j + 4
            yps = [psy.tile([D + 1, 512], FP, tag="y", name=f"yps_{c}_{j}_{k}")
                   for k in range(2)]
            for i in range(nkv):
                off = i - 4 * j
                q0 = max(0, 128 * off)
                spt = pss.tile([P, 1024], FP, tag="s")
                for hh in range(2):
                    p0 = 64 * hh
                    nc.tensor.matmul(
                        spt[:, hh * 512 + q0:(hh + 1) * 512],
                        lhsT=qk_sb[p0:p0 + 64, 4 + c, i * P:(i + 1) * P],
                        rhs=qk_sb[p0:p0 + 64, c, j * 512 + q0:(j + 1) * 512],
                        start=True, stop=True,
                        tile_position=(p0, 0))
                ptt = ptt_ring[ptt_ctr[0] % 4]
                ptt_ctr[0] += 1
                if off < 0:
                    nc.scalar.activation(ptt, spt, Exp, scale=float(SCALE))
                else:
                    pv = ptt.rearrange("p (h q) -> p h q", h=2)
                    sv = spt.rearrange("p (h q) -> p h q", h=2)
                    nc.scalar.activation(pv[:, :, q0:512], sv[:, :, q0:512],
                                         Exp, scale=float(SCALE))
                    # causal mask: zero stale cols [0,q0) + triangle
                    # [q0,q0+128) via mask constant (zeros(512)|tril(128))
                    for hx in range(2):
                        nc.vector.tensor_tensor(
                            pv[:, hx, 0:q0 + P], pv[:, hx, 0:q0 + P],
                            mask_sb[:, 512 - q0:640], MULT)
                for hh in range(2):
                    nc.tensor.matmul(
                        yps[hh],
                        lhsT=v_sb[:, i, 2 * c + hh, :],
                        rhs=ptt[:, hh * 512:(hh + 1) * 512],
                        start=(i == 0), stop=(i == nkv - 1))
                if fillers and (i + 1) % every == 0 and fillers:
                    fillers.pop(0)()
            # drain: y rows cast out, upper half DMA'd into yt NOW (not
            # norm-gated); denominator rows staged to partition 64 of stg
            # and DMA-shifted to a partition-0 tile for the norm chain
            # (custom-DVE/gpsimd ops mishandle nonzero base partitions).
            nc.vector.tensor_copy(yt_sb[0:D, c, jsl], yps[0][0:D, :])
            tmpb = drn.tile([D, 512], bf16, tag="tmpb", bufs=3)
            nc.vector.tensor_copy(tmpb, yps[1][0:D, :])
            nc.gpsimd.dma_start(yt_sb[64:128, c, jsl], tmpb)
            stg = drn.tile([D + 1, 1024], f32, tag="stg")
            nc.vector.tensor_copy(stg[D:D + 1, 0:512], yps[0][D:D + 1, :])
            nc.vector.tensor_copy(stg[D:D + 1, 512:1024], yps[1][D:D + 1, :])
            lp = nrm.tile([1, 1024], f32, tag="lp", bufs=4,
                          name=f"lp_{c}_{j}")
            nc.gpsimd.dma_start(lp, stg[D:D + 1, :])
            lps[(c, j)] = lp
            if dbgL_d is not None:
                nc.sync.dma_start(dbgL_d[4 * j + c], stg[D:D + 1, :])

        def norm(c, j, eng=None):
            """Normalize yt tile (c, j): two partition_broadcasts of the
            rcp rows + two in-place multiplies. Emitted ~a block after the
            drain so every dep is satisfied on arrival."""
            if norm_mode == "skip":
                return
            eng = eng or nc.vector
            jsl = slice(j * 512, (j + 1) * 512)
            lp = lps.pop((c, j))
            rc = nrm.tile([1, 1024], f32, tag="rc", bufs=2)
            nc.vector.reciprocal_approx_fast(rc, lp)
            rcb = nrm.tile([1, 1024], bf16, tag="rcb", bufs=2)
            nc.vector.tensor_copy(rcb, rc)
            bc = nrm.tile([P, 1024], bf16, tag="bc", bufs=3)
            nc.gpsimd.partition_broadcast(bc, rcb)
            if dbgB_d is not None:
                nc.sync.dma_start(dbgB_d[4 * j + c], rcb)
            eng.tensor_tensor(
                yt_sb[0:D, c, jsl], yt_sb[0:D, c, jsl], bc[0:D, 0:512], MULT)
            eng.tensor_tensor(
                yt_sb[D:P, c, jsl], yt_sb[D:P, c, jsl],
                bc[D:P, 512:1024], MULT)

        # ------------------- emission schedule -------------------
        # j=0 starts as soon as quarter-0 V chains + pair-0 QK chains are
        # in; remaining projections and (later) out-proj row tiles are
        # popped as fillers between chunks, norm(c,j) one block after its
        # drain. All exp-pacing, PE never starved.
        for tti in range(4):
            piece_v(0, tti)
        piece_qk(0, 0)
        piece_qk(0, 4)

        F = []
        for ft in (1, 5, 2, 6, 3, 7):
            F.append(lambda ft=ft: piece_qk(0, ft))
        for tti in range(4):
            F.append(lambda tti=tti: piece_v(1, tti))
        for ft in (0, 4, 1, 5, 2, 6, 3, 7):
            F.append(lambda ft=ft: piece_qk(1, ft))

        att_block(0, 0, F, every=1)
        att_block(1, 0, F, every=1)
        att_block(2, 0, F, every=1)
        att_block(3, 0, F, every=1)
        while F:
            F.pop(0)()

        # j=1: fillers = quarter-2 projections + norms of tile 0
        F = [lambda: norm(0, 0)]
        for tti in range(4):
            F.append(lambda tti=tti: piece_v(2, tti))
        F.append(lambda: norm(1, 0))
        for ft in (0, 4, 1, 5):
            F.append(lambda ft=ft: piece_qk(2, ft))
        F.append(lambda: norm(2, 0))
        for ft in (2, 6, 3, 7):
            F.append(lambda ft=ft: piece_qk(2, ft))
        F.append(lambda: norm(3, 0))

        att_block(0, 1, F, every=2)
        att_block(1, 1, F, every=2)
        att_block(2, 1, F, every=2)
        att_block(3, 1, F, every=2)
        while F:
            F.pop(0)()

        # j=2: fillers = quarter-3 projections + norms of tile 1 +
        # out-proj rows of quarter 0 (normalized during j=1)
        F = [lambda: norm(0, 1)]
        for tti in range(4):
            F.append(lambda tti=tti: piece_v(3, tti))
        F.append(lambda: norm(1, 1))
        for ft in (0, 4, 1, 5):
            F.append(lambda ft=ft: piece_qk(3, ft))
        F.append(lambda: norm(2, 1))
        for ft in (2, 6, 3, 7):
            F.append(lambda ft=ft: piece_qk(3, ft))
        F.append(lambda: norm(3, 1))

        att_block(0, 2, F, every=2)
        att_block(1, 2, F, every=2)
        # projections all emitted by here on the filler stream? force any
        # leftovers before swapping the PSUM pools
        att_block(2, 2, F, every=2)
        att_block(3, 2, F, every=2)
        while F:
            F.pop(0)()

        # projections done: swap the proj PSUM banks for the out-proj pool
        psp_ctx.close()
        pools["pso"] = top.enter_context(
            tc.tile_pool(name="pso", bufs=2, space="PSUM"))

        # j=3: fillers = norms of tile 2 + out-proj of quarters 0-2
        F = [lambda: norm(0, 2)]
        for tt in range(0, 2):
            for half in range(2):
                F.append(lambda tt=tt, half=half: piece_out(tt, half))
        F.append(lambda: norm(1, 2))
        for tt in range(2, 4):
            for half in range(2):
                F.append(lambda tt=tt, half=half: piece_out(tt, half))
        F.append(lambda: norm(2, 2))
        for tt in range(4, 6):
            for half in range(2):
                F.append(lambda tt=tt, half=half: piece_out(tt, half))
        F.append(lambda: norm(3, 2))
        for tt in range(6, 9):
            for half in range(2):
                F.append(lambda tt=tt, half=half: piece_out(tt, half))

        att_block(0, 3, F, every=2)
        F.append(lambda: norm(0, 3))
        for tt in range(9, 11):
            for half in range(2):
                F.append(lambda tt=tt, half=half: piece_out(tt, half))
        att_block(1, 3, F, every=2)
        F.append(lambda: norm(1, 3))
        for tt in range(11, 12):
            for half in range(2):
                F.append(lambda tt=tt, half=half: piece_out(tt, half))
        att_block(2, 3, F, every=2)
        F.append(lambda: norm(2, 3))
        att_block(3, 3, F, every=2)
        while F:
            F.pop(0)()
        norm(3, 3)
        for tt in range(12, 16):
            piece_out(tt, 0)
            piece_out(tt, 1)



def _shard_inputs(x, w_qkv, w_out):
    mask = np.zeros((P, 640), dtype=np.float32)
    mask[:, 512:640] = np.tril(np.ones((P, P), dtype=np.float32)).T
    mask = mask.astype(ml_dtypes.bfloat16)
    in_maps = []
    for core in range(8):
        b, hg = core // 2, core % 2
        sl = slice(hg * 512, (hg + 1) * 512)
        wq = w_qkv[0:1024][sl]
        wk = w_qkv[1024:2048][sl]
        wv = w_qkv[2048:3072][sl]
        wqkvT = np.concatenate([wq, wk, wv], axis=0).T  # [E, 1536]
        wvP = wqkvT[:, 1024:1536].reshape(NE, P, 512).transpose(1, 0, 2)
        wqkP = wqkvT[:, 0:1024].reshape(NE, P, 8, P).transpose(1, 2, 0, 3)
        woT = w_out[:, sl].T  # [512, E]
        woP = woT.reshape(4, P, E).transpose(1, 0, 2)
        xT = x[b].T.reshape(NE, P, T).transpose(1, 0, 2)  # [P, NE, T]
        cvt = lambda a: np.ascontiguousarray(a).astype(ml_dtypes.bfloat16)
        in_maps.append({
            "xT": cvt(xT),
            "wvP": cvt(wvP),
            "wqkP": cvt(wqkP),
            "woP": cvt(woP),
            "mask": mask,
        })
    return in_maps


def kernel(x, w_qkv, w_out, _trace=False):
    x = np.asarray(x, dtype=np.float32)
    w_qkv = np.asarray(w_qkv, dtype=np.float32)
    w_out = np.asarray(w_out, dtype=np.float32)

    if "nc" not in _CACHE:
        _CACHE["nc"] = build()
    nc = _CACHE["nc"]

    in_maps = _shard_inputs(x, w_qkv, w_out)
    res = bass_utils.run_bass_kernel_spmd(
        nc, in_maps, core_ids=list(range(8)), trace=_trace)
    kernel.last_result = res

    out = np.empty((B, T, E), dtype=np.float32)
    for b in range(B):
        out[b] = (res.results[2 * b]["out"].astype(np.float32)
                  + res.results[2 * b + 1]["out"].astype(np.float32))
    return out


# revision 37
# speedup vs baseline: 1.4116x; 1.1682x over previous
"""Causal self-attention Trainium2 kernel (B=4, T=2048, E=1024, H=16, D=64).

Sharding: 8 cores = batch(4) x head-group(2). Each core computes the full
attention for 8 heads of one batch element plus its half of the output
projection; the host sums the two out-proj partials per batch element.

v3 dataflow (per core, all matmul operands bf16, PSUM f32):
  - x^T [E,T] lives fully in SBUF, loaded with 4 large DMAs (one per
    T-quarter) on two queues so the first V-proj chain starts ~4us in.
  - Projection chains (V then QK per quarter) are interleaved into the
    attention stream as fillers so the PE never idles while ScalarE works
    through the exps; out-projection row tiles become fillers as soon as
    their query quarter is normalized.
  - S^T chunks [128kv, 2x512q]: two heads of a pair issued as row-group
    tiled matmul pairs (tile_position (0,0)/(64,0)) running concurrently.
    Diagonal chunks only compute live columns; exp on ScalarE -> bf16;
    causal mask via DVE multiply with a [128,640] zeros|tril constant.
  - y^T accumulation [65,512] per head, lhsT = V_aug (ones column carries
    the softmax denominator through the PV matmul).
  - Drain per (pair, tile): DVE casts y rows to yt/tmpb, tmpb is DMA'd
    into yt's upper partitions immediately (not norm-gated), DVE
    reciprocal runs directly on the PSUM denominator rows (partition 64),
    one cast + one tiny DMA lands both rcp rows on a partition-0 table.
  - Norm per (pair, tile), emitted ~one block later so every op's deps are
    met when it reaches its engine FIFO: 2 gpsimd partition_broadcasts +
    2 in-place DVE multiplies on yt. No cross-engine convoys.
  - ScalarE exp table is pre-warmed during the prologue DMAs.
"""

import numpy as np
import ml_dtypes

import concourse.bass as bass
import concourse.bacc as bacc
import concourse.mybir as mybir
import concourse.tile as tile
from concourse import bass_utils

f32 = mybir.dt.float32
bf16 = mybir.dt.bfloat16
FP = mybir.dt.float32  # psum dtype

P = 128
B, T, E = 4, 2048, 1024
H, D = 16, 64
HPC = H // 2            # heads per core = 8
NE = E // P             # 8 e-chunks
NTT = T // P            # 16 kv chunks
NQ = T // 512           # 4 query tiles of 512
SCALE = 1.0 / np.sqrt(D)

Exp = mybir.ActivationFunctionType.Exp
MULT = mybir.AluOpType.mult
IS_GE = mybir.AluOpType.is_ge

_CACHE = {}


def build(**opts):
    nc = bacc.Bacc("TRN2", target_bir_lowering=False, debug=False, num_devices=8)

    xT_d = nc.dram_tensor("xT", [P, NE, T], bf16, kind="ExternalInput")
    wvP_d = nc.dram_tensor("wvP", [P, NE, 512], bf16, kind="ExternalInput")
    wqkP_d = nc.dram_tensor("wqkP", [P, 8, NE, P], bf16, kind="ExternalInput")
    woP_d = nc.dram_tensor("woP", [P, 4, E], bf16, kind="ExternalInput")
    mask_d = nc.dram_tensor("mask", [P, P], bf16, kind="ExternalInput")
    out_d = nc.dram_tensor("out", [T, E], bf16, kind="ExternalOutput")
    dbgL_d = dbgB_d = None
    if opts.pop("debug_rcp", False):
        dbgL_d = nc.dram_tensor("dbgL", [16, 1024], f32,
                                kind="ExternalOutput")
        dbgB_d = nc.dram_tensor("dbgB", [16, 1024], bf16,
                                kind="ExternalOutput")

    with tile.TileContext(nc) as tc:
        build_body(tc, xT_d, wvP_d, wqkP_d, woP_d, mask_d, out_d,
                   dbgL_d=dbgL_d, dbgB_d=dbgB_d, **opts)
    nc.compile()
    return nc


def build_body(tc, xT_d, wvP_d, wqkP_d, woP_d, mask_d, out_d,
               pss_bufs=2, psy_bufs=2, norm_mode="full",
               dbgL_d=None, dbgB_d=None):
    nc = tc.nc

    from contextlib import ExitStack
    with ExitStack() as top:
        per = top.enter_context(tc.tile_pool(name="per", bufs=1))

        qk_sb = per.tile([P, 8, T], bf16)            # chunks 0-3: Q^T, 4-7: K^T
        v_sb = per.tile([P, NTT, HPC, D + 1], bf16)  # [kv_p, kv_chunk, head, d|1]
        yt_sb = per.tile([P, 4, T], bf16)            # [f%128, f//128, q]
        x_sb = per.tile([P, NE, T], bf16)            # x^T resident [p, e, t]
        wv_sb = per.tile([P, NE, 512], bf16)         # V-proj weights
        wqk_sb = per.tile([P, 8, NE, P], bf16)       # QK-proj weights per f-chunk
        wo_sb = per.tile([P, 4, E], bf16)            # out-proj weights
        mask_sb = per.tile([P, P], bf16)             # tril(128).T
        warm_sb = per.tile([4, 512], bf16)           # gpsimd ucode warmup dst
        # self-managed ptt ring: stale regions are never read (exp writes
        # [q0:512] per head and PV streams only those columns)
        ptt_ring = [per.tile([P, 1024], bf16, name=f"ptt{k}")
                    for k in range(4)]
        ptt_ctr = [0]

        # --------- prologue DMAs: 4 big x loads + weights, spread across
        # queues; warm the gpsimd ucode library and the ScalarE exp table
        # during the transfer ----------
        nc.gpsimd.partition_broadcast(warm_sb, ptt_ring[1][0:1, 0:512])
        # x quarter 0 first (sync queue) so V-proj can start asap; weights
        # spread across the sync/scalar queues so no single queue serializes
        # more than ~2MB ahead of the first consumers
        nc.sync.dma_start(x_sb[:, :, 0:512], xT_d[:, :, 0:512])
        nc.scalar.dma_start(wv_sb, wvP_d[:, :, :])
        nc.sync.dma_start(wqk_sb[:, 0:4], wqkP_d[:, 0:4])
        nc.scalar.dma_start(wqk_sb[:, 4:8], wqkP_d[:, 4:8])
        nc.scalar.dma_start(wo_sb, woP_d[:, :, :])
        nc.sync.dma_start(mask_sb, mask_d[:, :])
        # warm the exp table-set while DMAs fly (first ACT pays ~2.7us)
        nc.scalar.activation(warm_sb[:, 0:8], warm_sb[:, 0:8],
                             Exp, scale=1.0)
        for th in range(1, 4):
            nc.gpsimd.dma_start(
                x_sb[:, :, th * 512:(th + 1) * 512],
                xT_d[:, :, th * 512:(th + 1) * 512])

        # pool creation order: psp LAST so it sits on top of the PSUM stack
        # and can be swapped for the out-proj pool after projections end
        drn = top.enter_context(tc.tile_pool(name="drn", bufs=2))
        nrm = top.enter_context(tc.tile_pool(name="nrm", bufs=3))
        ost = top.enter_context(tc.tile_pool(name="ost", bufs=2))
        pss = top.enter_context(
            tc.tile_pool(name="pss", bufs=pss_bufs, space="PSUM"))
        psy = top.enter_context(
            tc.tile_pool(name="psy", bufs=psy_bufs, space="PSUM"))
        psp_ctx = ExitStack()
        psp = psp_ctx.enter_context(
            tc.tile_pool(name="psp", bufs=1, space="PSUM"))
        pools = {}
        done = set()
        lps = {}

        def piece_v(th, tti):
            # one V-projection chain: v_sb chunk tt, natural layout
            if ("v", th, tti) in done:
                return
            done.add(("v", th, tti))
            tt = th * 4 + tti
            ps = psp.tile([P, 1024], FP, tag="pq")
            for e in range(NE):
                nc.tensor.matmul(
                    ps[:, 0:512],
                    lhsT=x_sb[:, e, tt * P:(tt + 1) * P],
                    rhs=wv_sb[:, e, :],
                    start=(e == 0), stop=(e == NE - 1))
            nc.vector.tensor_copy(
                v_sb[:, tt, :, 0:D],
                ps[:, 0:512].rearrange("p (h d) -> p h d", h=HPC))
            if tti == 3:
                # ones column for this quarter (never keeps in_: cond<0)
                ov = v_sb[:, th * 4:(th + 1) * 4, :, D:D + 1]
                iv = v_sb[:, th * 4:(th + 1) * 4, :, 0:1]
                nc.gpsimd.affine_select(
                    ov, iv, pattern=[[0, 4], [0, HPC], [0, 1]],
                    compare_op=IS_GE, fill=1.0, base=-1,
                    channel_multiplier=0)

        def piece_qk(hf, ft):
            # one QK-projection chain over a T-half: each weight load
            # feeds two N=512 matmuls (adjacent quarters, same lhsT)
            if ("qk", hf, ft) in done:
                return
            done.add(("qk", hf, ft))
            h0 = hf * 1024
            ps = psp.tile([P, 1024], FP, tag="pq")
            for e in range(NE):
                nc.tensor.matmul(
                    ps[:, 0:512],
                    lhsT=wqk_sb[:, ft, e, :],
                    rhs=x_sb[:, e, h0:h0 + 512],
                    start=(e == 0), stop=(e == NE - 1))
                nc.tensor.matmul(
                    ps[:, 512:1024],
                    lhsT=wqk_sb[:, ft, e, :],
                    rhs=x_sb[:, e, h0 + 512:h0 + 1024],
                    start=(e == 0), stop=(e == NE - 1))
            nc.vector.tensor_copy(
                qk_sb[:, ft, h0:h0 + 1024], ps)

        def piece_out(tt):
            # one out-proj row tile, both halves: each yt weight load
            # feeds two N=512 matmuls into the two po PSUM buffers
            pso = pools["pso"]
            po0 = pso.tile([P, 512], FP, tag="po", name=f"po0_{tt}")
            po1 = pso.tile([P, 512], FP, tag="po", name=f"po1_{tt}")
            for c2 in range(4):
                nc.tensor.matmul(
                    po0,
                    lhsT=yt_sb[:, c2, tt * P:(tt + 1) * P],
                    rhs=wo_sb[:, c2, 0:512],
                    start=(c2 == 0), stop=(c2 == 3))
                nc.tensor.matmul(
                    po1,
                    lhsT=yt_sb[:, c2, tt * P:(tt + 1) * P],
                    rhs=wo_sb[:, c2, 512:1024],
                    start=(c2 == 0), stop=(c2 == 3))
            st = ost.tile([P, 1024], bf16, tag="st")
            nc.vector.tensor_copy(st[:, 0:512], po0)
            nc.vector.tensor_copy(st[:, 512:1024], po1)
            nc.sync.dma_start(out_d[tt * P:(tt + 1) * P, :], st)

        def att_block(c, j, fillers=None, every=4):
            """Attention for head pair (2c, 2c+1), query tile j.

            Pops one filler (independent work) every `every` chunks so the
            PE stays busy while ScalarE works through the exps."""
            jsl = slice(j * 512, (j + 1) * 512)
            nkv = 4 * j + 4
            yps = [psy.tile([D + 1, 512], FP, tag="y", name=f"yps_{c}_{j}_{k}")
                   for k in range(2)]
            for i in range(nkv):
                off = i - 4 * j
                q0 = max(0, 128 * off)
                spt = pss.tile([P, 1024], FP, tag="s")
                for hh in range(2):
                    p0 = 64 * hh
                    nc.tensor.matmul(
                        spt[:, hh * 512 + q0:(hh + 1) * 512],
                        lhsT=qk_sb[p0:p0 + 64, 4 + c, i * P:(i + 1) * P],
                        rhs=qk_sb[p0:p0 + 64, c, j * 512 + q0:(j + 1) * 512],
                        start=True, stop=True,
                        tile_position=(p0, 0))
                ptt = ptt_ring[ptt_ctr[0] % 4]
                ptt_ctr[0] += 1
                if off < 0:
                    nc.scalar.activation(ptt, spt, Exp, scale=float(SCALE))
                else:
                    pv = ptt.rearrange("p (h q) -> p h q", h=2)
                    sv = spt.rearrange("p (h q) -> p h q", h=2)
                    nc.scalar.activation(pv[:, :, q0:512], sv[:, :, q0:512],
                                         Exp, scale=float(SCALE))
                    # causal mask: zero the dead upper triangle of the
                    # [q0,q0+128) block; columns below q0 are never read
                    for hx in range(2):
                        nc.vector.tensor_tensor(
                            pv[:, hx, q0:q0 + P], pv[:, hx, q0:q0 + P],
                            mask_sb, MULT)
                for hh in range(2):
                    # stream only the live columns [q0:512]; dead columns
                    # keep earlier (sub-diagonal) partial sums, which is
                    # exact since dead chunks contribute zero there
                    nc.tensor.matmul(
                        yps[hh][:, q0:512],
                        lhsT=v_sb[:, i, 2 * c + hh, :],
                        rhs=ptt[:, hh * 512 + q0:(hh + 1) * 512],
                        start=(i == 0), stop=(i == nkv - 1),
                        skip_group_check=True)
                if fillers and (i + 1) % every == 0 and fillers:
                    fillers.pop(0)()
            # drain: y rows cast out, upper half DMA'd into yt NOW (not
            # norm-gated); denominator rows staged to partition 64 of stg
            # and DMA-shifted to a partition-0 tile for the norm chain
            # (custom-DVE/gpsimd ops mishandle nonzero base partitions).
            nc.vector.tensor_copy(yt_sb[0:D, c, jsl], yps[0][0:D, :])
            tmpb = drn.tile([D, 512], bf16, tag="tmpb", bufs=3)
            nc.vector.tensor_copy(tmpb, yps[1][0:D, :])
            nc.gpsimd.dma_start(yt_sb[64:128, c, jsl], tmpb)
            stg = drn.tile([D + 1, 1024], f32, tag="stg")
            nc.vector.tensor_copy(stg[D:D + 1, 0:512], yps[0][D:D + 1, :])
            nc.vector.tensor_copy(stg[D:D + 1, 512:1024], yps[1][D:D + 1, :])
            lp = nrm.tile([1, 1024], f32, tag="lp", bufs=4,
                          name=f"lp_{c}_{j}")
            nc.gpsimd.dma_start(lp, stg[D:D + 1, :])
            lps[(c, j)] = lp
            if dbgL_d is not None:
                nc.sync.dma_start(dbgL_d[4 * j + c], stg[D:D + 1, :])

        def norm(c, j, eng=None):
            """Normalize yt tile (c, j): two partition_broadcasts of the
            rcp rows + two in-place multiplies. Emitted ~a block after the
            drain so every dep is satisfied on arrival."""
            if norm_mode == "skip":
                return
            eng = eng or nc.vector
            jsl = slice(j * 512, (j + 1) * 512)
            lp = lps.pop((c, j))
            rc = nrm.tile([1, 1024], f32, tag="rc", bufs=2)
            nc.vector.reciprocal_approx_fast(rc, lp)
            rcb = nrm.tile([1, 1024], bf16, tag="rcb", bufs=2)
            nc.vector.tensor_copy(rcb, rc)
            bc = nrm.tile([P, 1024], bf16, tag="bc", bufs=3)
            nc.gpsimd.partition_broadcast(bc, rcb)
            if dbgB_d is not None:
                nc.sync.dma_start(dbgB_d[4 * j + c], rcb)
            eng.tensor_tensor(
                yt_sb[0:D, c, jsl], yt_sb[0:D, c, jsl], bc[0:D, 0:512], MULT)
            eng.tensor_tensor(
                yt_sb[D:P, c, jsl], yt_sb[D:P, c, jsl],
                bc[D:P, 512:1024], MULT)

        # ------------------- emission schedule -------------------
        # j=0 starts as soon as quarter-0 V chains + pair-0 QK chains are
        # in; remaining projections and (later) out-proj row tiles are
        # popped as fillers between chunks, norm(c,j) one block after its
        # drain. All exp-pacing, PE never starved.
        for tti in range(4):
            piece_v(0, tti)
        piece_qk(0, 0)
        piece_qk(0, 4)

        F = []
        for ft in (1, 5, 2, 6, 3, 7):
            F.append(lambda ft=ft: piece_qk(0, ft))
        for tti in range(4):
            F.append(lambda tti=tti: piece_v(1, tti))

        att_block(0, 0, F, every=1)
        att_block(1, 0, F, every=1)
        att_block(2, 0, F, every=1)
        att_block(3, 0, F, every=1)
        while F:
            F.pop(0)()

        # j=1: fillers = quarter-2 V chains + T-half-1 QK chains (Q tiles
        # 2-3 + K quarters 2-3) + norms of tile 0
        F = [lambda: norm(0, 0)]
        for tti in range(4):
            F.append(lambda tti=tti: piece_v(2, tti))
        F.append(lambda: norm(1, 0))
        for ft in (0, 4, 1, 5):
            F.append(lambda ft=ft: piece_qk(1, ft))
        F.append(lambda: norm(2, 0))
        F.append(lambda: norm(3, 0))

        att_block(0, 1, F, every=2)
        att_block(1, 1, F, every=2)
        att_block(2, 1, F, every=2)
        att_block(3, 1, F, every=2)
        while F:
            F.pop(0)()

        # j=2: fillers = quarter-3 V chains + remaining QK + norms of tile 1
        F = [lambda: norm(0, 1)]
        for tti in range(4):
            F.append(lambda tti=tti: piece_v(3, tti))
        F.append(lambda: norm(1, 1))
        for ft in (2, 6, 3, 7):
            F.append(lambda ft=ft: piece_qk(1, ft))
        F.append(lambda: norm(2, 1))
        F.append(lambda: norm(3, 1))

        att_block(0, 2, F, every=2)
        att_block(1, 2, F, every=2)
        att_block(2, 2, F, every=2)
        att_block(3, 2, F, every=2)
        while F:
            F.pop(0)()

        # projections done: swap the proj PSUM banks for the out-proj pool
        psp_ctx.close()
        pools["pso"] = top.enter_context(
            tc.tile_pool(name="pso", bufs=2, space="PSUM"))

        # j=3: fillers = norms of tile 2 + out-proj of quarters 0-2
        F = [lambda: norm(0, 2)]
        F.append(lambda: piece_out(0))
        F.append(lambda: norm(1, 2))
        for tt in (1, 2):
            F.append(lambda tt=tt: piece_out(tt))
        F.append(lambda: norm(2, 2))
        for tt in (3, 4):
            F.append(lambda tt=tt: piece_out(tt))
        F.append(lambda: norm(3, 2))
        for tt in (5, 6, 7, 8):
            F.append(lambda tt=tt: piece_out(tt))

        att_block(0, 3, F, every=2)
        F.append(lambda: norm(0, 3))
        for tt in (9, 10):
            F.append(lambda tt=tt: piece_out(tt))
        att_block(1, 3, F, every=2)
        F.append(lambda: norm(1, 3))
        F.append(lambda: piece_out(11))
        att_block(2, 3, F, every=2)
        F.append(lambda: norm(2, 3))
        att_block(3, 3, F, every=2)
        while F:
            F.pop(0)()
        norm(3, 3)
        for tt in range(12, 16):
            piece_out(tt)



def _shard_inputs(x, w_qkv, w_out):
    mask = np.tril(np.ones((P, P), dtype=np.float32)).T
    mask = mask.astype(ml_dtypes.bfloat16)
    in_maps = []
    for core in range(8):
        b, hg = core // 2, core % 2
        sl = slice(hg * 512, (hg + 1) * 512)
        wq = w_qkv[0:1024][sl]
        wk = w_qkv[1024:2048][sl]
        wv = w_qkv[2048:3072][sl]
        wqkvT = np.concatenate([wq, wk, wv], axis=0).T  # [E, 1536]
        wvP = wqkvT[:, 1024:1536].reshape(NE, P, 512).transpose(1, 0, 2)
        wqkP = wqkvT[:, 0:1024].reshape(NE, P, 8, P).transpose(1, 2, 0, 3)
        woT = w_out[:, sl].T  # [512, E]
        woP = woT.reshape(4, P, E).transpose(1, 0, 2)
        xT = x[b].T.reshape(NE, P, T).transpose(1, 0, 2)  # [P, NE, T]
        cvt = lambda a: np.ascontiguousarray(a).astype(ml_dtypes.bfloat16)
        in_maps.append({
            "xT": cvt(xT),
            "wvP": cvt(wvP),
            "wqkP": cvt(wqkP),
            "woP": cvt(woP),
            "mask": mask,
        })
    return in_maps


def kernel(x, w_qkv, w_out, _trace=False):
    x = np.asarray(x, dtype=np.float32)
    w_qkv = np.asarray(w_qkv, dtype=np.float32)
    w_out = np.asarray(w_out, dtype=np.float32)

    if "nc" not in _CACHE:
        _CACHE["nc"] = build()
    nc = _CACHE["nc"]

    in_maps = _shard_inputs(x, w_qkv, w_out)
    res = bass_utils.run_bass_kernel_spmd(
        nc, in_maps, core_ids=list(range(8)), trace=_trace)
    kernel.last_result = res

    out = np.empty((B, T, E), dtype=np.float32)
    for b in range(B):
        out[b] = (res.results[2 * b]["out"].astype(np.float32)
                  + res.results[2 * b + 1]["out"].astype(np.float32))
    return out
